# revision 7
# baseline (speedup 1.0000x reference)
"""Trainium2 Bass kernel for nn_Attention_48095043781121 (v2).

Math (reference):
    q,k,v = x@Wq, x@Wk, x@Wv          (per head h: columns [64h, 64h+64))
    A     = softmax_j(q.k^T / 8)
    p     = relu(pos@Wp1+bp1)@Wp2+bp2
    P[b,h,i,j] = softmax_j(ph_i - ph_j + bh) = softmax_j(-ph_j) = w[b,h,j]
                 (i-part, bh AND the bp2 contribution all cancel in softmax)
    attn  = ((1-g)A + gP) / rowsum               rowsum == 1 exactly
    out   = attn @ v ;  y = concat_heads(out) @ Wo + bo

Per (b,h):  y += [(1-g_h)/r] * (E @ v_h) @ Wo_h  +  [g_h * (w @ v_h)] @ Wo_h
with E = exp(S/8), r[i] = sum_j E[i,j].  The second term is a constant row
(independent of the query i) -> computed once as `yb` and added on the host
along with bo during the unshard.

Sharding: 8 cores = 4 batches x 2 head-groups (heads 0-3 / 4-7); host sums
the two partial y (+ yb rows + bo) per batch.

v2 layout: everything fp16 on the PE.  E is stored [j-part, i-free]; the
E@v matmul uses v (augmented with a 1/(1-g) column) as the stationary
operand and E as the moving operand, so the output lands directly in
[feature-part, token-free] orientation for the out-projection -- no PE
transposes.  Row 64 of each head's output is r/(1-g); its reciprocal is
broadcast over 64 partitions with a K=1 fp32r matmul and multiplied in
(partition-shifted writes stack the odd head into rows 64-127).
"""

import numpy as np
from contextlib import ExitStack

B, S, DIM, H, DH = 4, 512, 512, 8, 64
POS_DIM, PD8 = 3, 64
NCORES = 8
HGH = 4          # heads per head-group (per core)
HGF = HGH * DH   # feature columns per head-group = 256
KT = DIM // 128  # contraction tiles over model dim = 4
MT = HGF // 128  # feature tiles per head-group = 2
ST = S // 128    # token tiles = 4
DHA = DH + 1     # v columns padded: [v(64) | 1/(1-g)]
NWARM = 9        # HAM warmup matmuls bridging the input-DMA head

_CACHE = {}


def _build_program():
    import concourse.mybir as mybir
    import concourse.tile as tile
    from concourse import bacc
    from concourse.masks import make_identity

    F32 = mybir.dt.float32
    F32R = mybir.dt.float32r
    F16 = mybir.dt.float16
    AF = mybir.ActivationFunctionType
    ALU = mybir.AluOpType

    nc = bacc.Bacc(trn_type="TRN2", target_bir_lowering=False, debug=False)

    xT_d = nc.dram_tensor("xT", [128, KT * S], F16, kind="ExternalInput")
    wq_d = nc.dram_tensor("Wq", [128, KT * HGF], F16, kind="ExternalInput")
    wk_d = nc.dram_tensor("Wk", [128, KT * HGF], F16, kind="ExternalInput")
    wv_d = nc.dram_tensor("Wv", [128, KT * HGF], F16, kind="ExternalInput")
    wo_d = nc.dram_tensor("Wo", [128, MT * DIM], F16, kind="ExternalInput")
    # posP: [posT(512) | Wp1 padded to 4 | Wp2(64)]
    posP_d = nc.dram_tensor("posP", [POS_DIM, S + 4 + PD8], F32R, kind="ExternalInput")
    whP_d = nc.dram_tensor("whP", [PD8, HGH], F32R, kind="ExternalInput")
    # sclP: [bp1 | g]
    sclP_d = nc.dram_tensor("sclP", [HGH, 2], F32, kind="ExternalInput")
    vpad_d = nc.dram_tensor("vpad", [128, ST * HGH], F16, kind="ExternalInput")
    y_d = nc.dram_tensor("y", [S, DIM], F16, kind="ExternalOutput")
    yb_d = nc.dram_tensor("yb", [1, DIM], F32, kind="ExternalOutput")

    with tile.TileContext(nc) as tc, ExitStack() as ctx:
        sing = ctx.enter_context(tc.tile_pool(name="sing", bufs=1))
        scpool = ctx.enter_context(tc.tile_pool(name="scpool", bufs=2))
        ypool = ctx.enter_context(tc.tile_pool(name="ypool", bufs=2))
        ps_mm = ctx.enter_context(tc.tile_pool(name="ps_mm", bufs=3, space="PSUM"))
        ps_o = ctx.enter_context(tc.tile_pool(name="ps_o", bufs=3, space="PSUM"))
        ps_sel = ctx.enter_context(tc.tile_pool(name="ps_sel", bufs=1, space="PSUM"))
        ps_t = ctx.enter_context(tc.tile_pool(name="ps_t", bufs=1, space="PSUM"))

        # ---------------- input DMAs (4 queues) ----------------
        xT = sing.tile([128, KT, S], F16)
        wq = sing.tile([128, KT, HGF], F16)
        wk = sing.tile([128, KT, HGF], F16)
        wv = sing.tile([128, KT, HGF], F16)
        wo = sing.tile([128, MT, DIM], F16)
        v_aug = sing.tile([128, ST, HGH, DHA], F16)
        posP = sing.tile([POS_DIM, S + 4 + PD8], F32R)
        whP = sing.tile([PD8, HGH], F32R)
        sclP = sing.tile([HGH, 2], F32)

        xT_r = xT_d.ap()
        nc.sync.dma_start(out=xT[:, 0:2, :], in_=xT_r[:, 0 : 2 * S])
        nc.sync.dma_start(out=wk, in_=wk_d.ap())
        nc.scalar.dma_start(out=wq, in_=wq_d.ap())
        nc.scalar.dma_start(out=xT[:, 2:KT, :], in_=xT_r[:, 2 * S : KT * S])
        nc.gpsimd.dma_start(out=posP, in_=posP_d.ap())
        nc.gpsimd.dma_start(out=whP, in_=whP_d.ap())
        nc.gpsimd.dma_start(out=sclP, in_=sclP_d.ap())
        # aug column of v (1/(1-g) per head) straight from the host
        nc.gpsimd.dma_start(
            out=v_aug[:, :, :, DH : DH + 1],
            in_=vpad_d.ap().rearrange("p (t h) -> p t h", h=HGH)[:, :, :, None],
        )
        nc.gpsimd.dma_start(out=wv, in_=wv_d.ap())
        nc.gpsimd.dma_start(out=wo, in_=wo_d.ap())

        # ---------------- constants + HAM warmup ----------------
        warm = sing.tile([128, 512], F16)
        nc.vector.memset(warm, 0.25)
        ones1_f = sing.tile([1, DH], F32)
        nc.vector.memset(ones1_f, 1.0)
        ones64r = sing.tile([1, DH], F32R)
        nc.vector.tensor_copy(ones64r, ones1_f)
        ident = sing.tile([128, 128], F32)
        make_identity(nc, ident)
        with nc.named_scope("warmup"):
            for _ in range(NWARM):
                wps = ps_mm.tile([128, 512], F32, tag="mm")
                nc.tensor.matmul(wps, warm[:, 0:128], warm, start=True, stop=True)

        # ---------------- position path ----------------
        with nc.named_scope("pos_path"):
            p1ps = ps_mm.tile([4, S], F32, tag="mm")
            nc.tensor.matmul(
                p1ps, posP[:, S : S + 4], posP[:, 0:S], start=True, stop=True
            )
            p1 = sing.tile([4, S], F32R)
            nc.scalar.activation(p1, p1ps, AF.Relu, bias=sclP[:, 0:1])
            p2ps = ps_mm.tile([PD8, S], F32, tag="mm")
            nc.tensor.matmul(
                p2ps, posP[:, S + 4 :], p1[0:POS_DIM, :], start=True, stop=True
            )
            p2 = sing.tile([PD8, S], F32R)
            nc.vector.tensor_copy(p2, p2ps)  # bp2 cancels in the softmax
            phps = ps_mm.tile([HGH, S], F32, tag="mm")
            nc.tensor.matmul(phps, whP, p2, start=True, stop=True)
            wexp = sing.tile([HGH, S], F32)
            wsum = sing.tile([HGH, 1], F32)
            nc.scalar.activation(wexp, phps, AF.Exp, scale=-1.0, accum_out=wsum)
            winv = sing.tile([HGH, 1], F32)
            nc.vector.reciprocal(winv, wsum)
            gwin = sing.tile([HGH, 1], F32)
            nc.vector.tensor_mul(gwin, winv, sclP[:, 1:2])
            w_sb = sing.tile([HGH, S], F32)
            nc.vector.tensor_scalar_mul(w_sb, wexp, gwin)
            # w as columns for the w@v contraction
            wj = sing.tile([128, ST, HGH], F16)
            for jt in range(ST):
                wt = ps_t.tile([128, HGH], F32, tag="t")
                nc.tensor.transpose(
                    wt, w_sb[:, 128 * jt : 128 * (jt + 1)], ident[0:HGH, 0:HGH]
                )
                nc.vector.tensor_copy(wj[:, jt, :], wt)

        # ---------------- projections ----------------
        kT_sb = sing.tile([128, MT, S], F16)
        qT_sb = sing.tile([128, MT, S], F16)
        with nc.named_scope("proj_kq"):
            for m in range(MT):
                for dst, w in ((kT_sb, wk), (qT_sb, wq)):
                    ps = ps_mm.tile([128, S], F32, tag="mm")
                    for kk in range(KT):
                        nc.tensor.matmul(
                            ps,
                            w[:, kk, 128 * m : 128 * (m + 1)],
                            xT[:, kk, :],
                            start=(kk == 0),
                            stop=(kk == KT - 1),
                        )
                    nc.scalar.activation(dst[:, m, :], ps, AF.Copy)

        with nc.named_scope("proj_v"):
            for tt in range(ST):
                ps = ps_mm.tile([128, HGF], F32, tag="mm")
                for kk in range(KT):
                    nc.tensor.matmul(
                        ps,
                        xT[:, kk, 128 * tt : 128 * (tt + 1)],
                        wv[:, kk, :],
                        start=(kk == 0),
                        stop=(kk == KT - 1),
                    )
                nc.vector.tensor_copy(
                    v_aug[:, tt, :, 0:DH],
                    ps.rearrange("p (h c) -> p h c", c=DH),
                )

        # ---------------- attention ----------------
        e_sb = sing.tile([128, HGH, ST, S], F16)
        oT = sing.tile([128, MT, S], F16)
        gwv_cols = sing.tile([128, MT], F16)
        srows = [sing.tile([1, S], F32R, name=f"srow{h}") for h in range(HGH)]
        ups = [None] * HGH

        def score(h):
            m, sub = h // 2, h % 2
            off = 64 * sub
            for jt in range(ST):
                sps = ps_mm.tile([128, S], F32, tag="mm")
                nc.tensor.matmul(
                    sps,
                    kT_sb[off : off + 64, m, 128 * jt : 128 * (jt + 1)],
                    qT_sb[off : off + 64, m, :],
                    start=True,
                    stop=True,
                )
                nc.scalar.activation(e_sb[:, h, jt, :], sps, AF.Exp, scale=0.125)

        def mm2(h):
            u = ps_o.tile([DHA, S], F32, tag="o")
            ups[h] = u
            for jt in range(ST):
                nc.tensor.matmul(
                    u,
                    v_aug[:, jt, h, :],
                    e_sb[:, h, jt, :],
                    start=(jt == 0),
                    stop=(jt == ST - 1),
                )
            with nc.allow_low_precision(reason="f32r stores identical fp32 bits"):
                nc.vector.reciprocal(srows[h], u[DH : DH + 1, :])

        def gwv(h):
            gw = ps_t.tile([DH, 1], F32, tag="t")
            for jt in range(ST):
                nc.tensor.matmul(
                    gw,
                    v_aug[:, jt, h, 0:DH],
                    wj[:, jt, h : h + 1],
                    start=(jt == 0),
                    stop=(jt == ST - 1),
                )
            off = 64 * (h % 2)
            nc.vector.tensor_copy(gwv_cols[off : off + 64, h // 2 : h // 2 + 1], gw)

        def sel_combine(h):
            sc_ps = ps_sel.tile([DH, S], F32, tag="sel")
            nc.tensor.matmul(sc_ps, ones64r, srows[h], start=True, stop=True)
            scSB = scpool.tile([DH, S], F32, tag="sc")
            nc.vector.tensor_copy(scSB, sc_ps)
            off = 64 * (h % 2)
            nc.vector.tensor_tensor(
                out=oT[off : off + 64, h // 2, :],
                in0=ups[h][0:DH, :],
                in1=scSB,
                op=ALU.mult,
            )

        with nc.named_scope("attn"):
            score(0)
            score(1)
            for h in range(HGH):
                gwv(h)
            mm2(0)
            score(2)
            mm2(1)
            sel_combine(0)
            score(3)
            mm2(2)
            sel_combine(1)
            mm2(3)
            sel_combine(2)
            sel_combine(3)

        # ---------------- ybias row: sum_h g*(w@v_h) @ Wo_h ----------------
        with nc.named_scope("ybias"):
            yb_ps = ps_sel.tile([1, DIM], F32, tag="sel")
            for m in range(MT):
                nc.tensor.matmul(
                    yb_ps,
                    gwv_cols[:, m : m + 1],
                    wo[:, m, :],
                    start=(m == 0),
                    stop=(m == MT - 1),
                )
            ybsb = sing.tile([1, DIM], F32)
            nc.vector.tensor_copy(ybsb, yb_ps)
            nc.sync.dma_start(out=yb_d.ap(), in_=ybsb)

        # ---------------- out-projection ----------------
        with nc.named_scope("outproj"):
            yps = [None] * ST
            for it in range(ST):
                yps[it] = ps_o.tile([128, DIM], F32, tag="o", name=f"yps{it}")
                nc.tensor.matmul(
                    yps[it],
                    oT[:, 0, 128 * it : 128 * (it + 1)],
                    wo[:, 0, :],
                    start=True,
                    stop=False,
                )
            for it in range(ST):
                nc.tensor.matmul(
                    yps[it],
                    oT[:, 1, 128 * it : 128 * (it + 1)],
                    wo[:, 1, :],
                    start=False,
                    stop=True,
                )
                ysb = ypool.tile([128, DIM], F16, tag="y")
                nc.scalar.activation(ysb, yps[it], AF.Copy)
                nc.sync.dma_start(
                    out=y_d.ap()[128 * it : 128 * (it + 1), :], in_=ysb
                )

    nc.compile()
    return nc


def _get_program():
    if "nc" not in _CACHE:
        _CACHE["nc"] = _build_program()
    return _CACHE["nc"]


def _ktile(a, dtype=np.float16):
    # [K*128, n] -> [128, K*n] (per-partition-contiguous k-tile layout)
    k = a.shape[0] // 128
    return np.ascontiguousarray(
        a.reshape(k, 128, a.shape[1]).transpose(1, 0, 2).reshape(128, -1).astype(dtype)
    )


def _make_in_maps(inputs):
    f = lambda a: np.ascontiguousarray(np.asarray(a), dtype=np.float32)
    x = f(inputs["x"])
    pos = f(inputs["pos"])
    Wq, Wk, Wv, Wo = f(inputs["Wq"]), f(inputs["Wk"]), f(inputs["Wv"]), f(inputs["Wo"])
    Wp1, bp1 = f(inputs["Wp1"]), f(inputs["bp1"])
    Wh, gate = f(inputs["Wh"]), f(inputs["gate"])
    gfull = 1.0 / (1.0 + np.exp(-gate.astype(np.float64)))  # sigmoid on host

    wp1_pad = np.zeros((POS_DIM, 4), np.float32)
    wp1_pad[:, :POS_DIM] = Wp1
    bp1_pad = np.zeros((HGH,), np.float32)
    bp1_pad[:POS_DIM] = bp1
    Wp2 = f(inputs["Wp2"])  # [3, 64]; bp2 cancels in the softmax

    in_maps = []
    for c in range(NCORES):
        b, hg = c // 2, c % 2
        cs = slice(HGF * hg, HGF * (hg + 1))
        g = gfull[HGH * hg : HGH * (hg + 1)].astype(np.float32)
        inv1mg = (1.0 / (1.0 - g.astype(np.float64))).astype(np.float32)
        posP = np.concatenate(
            [np.ascontiguousarray(pos[b].T), wp1_pad, Wp2], axis=1
        ).astype(np.float32)
        sclP = np.stack([bp1_pad, np.pad(g, (0, HGH - HGH))], axis=1)
        sclP = np.zeros((HGH, 2), np.float32)
        sclP[:, 0] = bp1_pad
        sclP[:, 1] = g
        vpad = np.tile(inv1mg.astype(np.float16)[None, :], (128, ST)).reshape(128, -1)
        in_maps.append(
            {
                "xT": _ktile(x[b].T),
                "Wq": _ktile(Wq[:, cs]),
                "Wk": _ktile(Wk[:, cs]),
                "Wv": _ktile(Wv[:, cs]),
                "Wo": _ktile(Wo[cs, :]),
                "posP": posP,
                "whP": np.ascontiguousarray(Wh[:, HGH * hg : HGH * (hg + 1)]),
                "sclP": sclP,
                "vpad": np.ascontiguousarray(vpad),
            }
        )
    return in_maps


def run(inputs, trace=False):
    """Run on 8 NeuronCores; returns (out [B,S,DIM] fp32, BassKernelResults)."""
    from concourse.bass_utils import run_bass_kernel_spmd

    nc = _get_program()
    in_maps = _make_in_maps(inputs)
    res = run_bass_kernel_spmd(
        nc, in_maps, core_ids=list(range(NCORES)), trace=trace
    )
    bo = np.asarray(inputs["bo"], np.float32)
    out = np.empty((B, S, DIM), np.float32)
    for b in range(B):
        r0, r1 = res.results[2 * b], res.results[2 * b + 1]
        out[b] = (
            r0["y"].astype(np.float32)
            + r1["y"].astype(np.float32)
            + r0["yb"]
            + r1["yb"]
            + bo[None, :]
        )
    return out, res


def kernel(**inputs):
    out, _ = run(inputs, trace=False)
    return out


# revision 14
# speedup vs baseline: 1.2218x; 1.2218x over previous
"""Trainium2 Bass kernel for nn_Attention_48095043781121 (v3).

Math (reference):
    q,k,v = x@Wq, x@Wk, x@Wv          (per head h: columns [64h, 64h+64))
    A     = softmax_j(q.k^T / 8)
    p     = relu(pos@Wp1+bp1)@Wp2+bp2
    P[b,h,i,j] = softmax_j(ph_i - ph_j + bh) = softmax_j(-ph_j) = w[b,h,j]
                 (i-part, bh AND the bp2 contribution all cancel in softmax)
    attn  = ((1-g)A + gP) / rowsum               rowsum == 1 exactly
    out   = attn @ v ;  y = concat_heads(out) @ Wo + bo

Per (b,h):  y += [(1-g_h)/r] * (E @ v_h) @ Wo_h  +  [g_h * (w @ v_h)] @ Wo_h
with E = exp(S/8), r[i] = sum_j E[i,j].  The second term is a constant row
(independent of the query i) -> computed once as `yb` and added on the host
along with bo during the unshard.

Sharding: 8 cores = 4 batches x 2 head-groups (heads 0-3 / 4-7); host sums
the two partial y (+ yb rows + bo) per batch.

v3 structure (all fp16 on the PE):
  - E stored [j-part, i-free]; E@v uses v (augmented with a 1/(1-g) column)
    as stationary and E as moving, so output lands [feature-part, i-free]
    for the out-projection -- no PE transposes.
  - Scores for a head pair run as two concurrent row-group matmuls
    (K=64 at partitions 0-63 / 64-127) into the two banks of one
    [128, 2, 512] PSUM pair tile; ONE exp ACTIVATE covers the pair
    (amortizes the ~260ns ACT per-op overhead).
  - Row 64 of each head's E@v is r/(1-g); reciprocal_approx_fast + a K=1
    fp32r matmul broadcasts (1-g)/r over 64 partitions; a partition-shifted
    DVE multiply stacks the odd head into oT rows 64-127.
  - The pos-MLP runs inside the exp-bound attention window.
"""

import numpy as np
from contextlib import ExitStack

B, S, DIM, H, DH = 4, 512, 512, 8, 64
POS_DIM, PD8 = 3, 64
NCORES = 8
HGH = 4          # heads per head-group (per core)
HGF = HGH * DH   # feature columns per head-group = 256
KT = DIM // 128  # contraction tiles over model dim = 4
MT = HGF // 128  # feature tiles per head-group = 2
ST = S // 128    # token tiles = 4
DHA = DH + 1     # v columns padded: [v(64) | 1/(1-g)]
NWARM = 11       # HAM warmup matmuls bridging the input-DMA head

_CACHE = {}


def _build_program():
    import concourse.mybir as mybir
    import concourse.tile as tile
    from concourse import bacc
    from concourse.masks import make_identity

    F32 = mybir.dt.float32
    F32R = mybir.dt.float32r
    F16 = mybir.dt.float16
    AF = mybir.ActivationFunctionType
    ALU = mybir.AluOpType

    nc = bacc.Bacc(trn_type="TRN2", target_bir_lowering=False, debug=False)

    xT_d = nc.dram_tensor("xT", [128, KT * S], F16, kind="ExternalInput")
    wq_d = nc.dram_tensor("Wq", [128, KT * HGF], F16, kind="ExternalInput")
    wk_d = nc.dram_tensor("Wk", [128, KT * HGF], F16, kind="ExternalInput")
    wv_d = nc.dram_tensor("Wv", [128, KT * HGF], F16, kind="ExternalInput")
    wo_d = nc.dram_tensor("Wo", [128, MT * DIM], F16, kind="ExternalInput")
    # posP: [posT(512) | Wp1 padded to 4 | Wp2(64)]
    posP_d = nc.dram_tensor("posP", [POS_DIM, S + 4 + PD8], F32R, kind="ExternalInput")
    whP_d = nc.dram_tensor("whP", [PD8, HGH], F32R, kind="ExternalInput")
    # sclP: [bp1 | g]
    sclP_d = nc.dram_tensor("sclP", [HGH, 2], F32, kind="ExternalInput")
    vpad_d = nc.dram_tensor("vpad", [128, ST * HGH], F16, kind="ExternalInput")
    y_d = nc.dram_tensor("y", [S, DIM], F16, kind="ExternalOutput")
    yb_d = nc.dram_tensor("yb", [1, DIM], F32, kind="ExternalOutput")

    with tile.TileContext(nc) as tc, ExitStack() as ctx:
        sing = ctx.enter_context(tc.tile_pool(name="sing", bufs=1))
        scpool = ctx.enter_context(tc.tile_pool(name="scpool", bufs=2))
        ypool = ctx.enter_context(tc.tile_pool(name="ypool", bufs=2))
        # PSUM: 8 banks = ps_big 2x2 + ps_o 2x1 + ps_sel 2x1
        ps_big = ctx.enter_context(tc.tile_pool(name="ps_big", bufs=2, space="PSUM"))
        ps_o = ctx.enter_context(tc.tile_pool(name="ps_o", bufs=2, space="PSUM"))
        ps_sel = ctx.enter_context(tc.tile_pool(name="ps_sel", bufs=2, space="PSUM"))

        # ---------------- input DMAs (3 queues) ----------------
        xT = sing.tile([128, KT, S], F16)
        wq = sing.tile([128, KT, HGF], F16)
        wk = sing.tile([128, KT, HGF], F16)
        wv = sing.tile([128, KT, HGF], F16)
        wo = sing.tile([128, MT, DIM], F16)
        v_aug = sing.tile([128, ST, HGH, DHA], F16)
        posP = sing.tile([POS_DIM, S + 4 + PD8], F32R)
        whP = sing.tile([PD8, HGH], F32R)
        sclP = sing.tile([HGH, 2], F32)

        xT_r = xT_d.ap()
        nc.sync.dma_start(out=xT[:, 0:2, :], in_=xT_r[:, 0 : 2 * S])
        nc.scalar.dma_start(out=wq, in_=wq_d.ap())
        nc.scalar.dma_start(out=xT[:, 2:KT, :], in_=xT_r[:, 2 * S : KT * S])
        nc.gpsimd.dma_start(out=wk, in_=wk_d.ap())
        nc.gpsimd.dma_start(out=wv, in_=wv_d.ap())
        nc.gpsimd.dma_start(out=posP, in_=posP_d.ap())
        nc.gpsimd.dma_start(out=whP, in_=whP_d.ap())
        nc.gpsimd.dma_start(out=sclP, in_=sclP_d.ap())
        # aug column of v (1/(1-g) per head) straight from the host
        nc.gpsimd.dma_start(
            out=v_aug[:, :, :, DH : DH + 1],
            in_=vpad_d.ap().rearrange("p (t h) -> p t h", h=HGH)[:, :, :, None],
        )
        nc.gpsimd.dma_start(out=wo, in_=wo_d.ap())

        # ---------------- constants + HAM warmup ----------------
        warm = sing.tile([128, 512], F16)
        nc.vector.memset(warm, 0.25)
        ones1_f = sing.tile([1, DH], F32)
        nc.vector.memset(ones1_f, 1.0)
        ones64h = sing.tile([1, DH], F16)
        nc.vector.tensor_copy(ones64h, ones1_f)
        ident = sing.tile([128, 128], F32)
        make_identity(nc, ident)
        with nc.named_scope("warmup"):
            for _ in range(NWARM):
                wps = ps_big.tile([128, 512], F32, tag="big")
                nc.tensor.matmul(wps, warm[:, 0:128], warm, start=True, stop=True)

        # ---------------- projections / scores / attention ----------------
        # kqT[:, m, 0, :] = k features (m-block), kqT[:, m, 1, :] = q
        kqT = sing.tile([128, MT, 2, S], F16)
        e_sb = sing.tile([128, ST, HGH, S], F16)
        oT = sing.tile([128, MT, S], F16)
        gwv_cols = sing.tile([128, MT], F16)
        srows = [sing.tile([1, S], F16, name=f"srow{h}") for h in range(HGH)]
        srowFs = [sing.tile([1, S], F32, name=f"srowF{h}") for h in range(HGH)]
        rrows = [sing.tile([1, S], F32, name=f"rrow{h}") for h in range(HGH)]
        ups = [None] * HGH

        def proj_kq(m):
            pair = ps_big.tile([128, 2, S], F32, tag="big", name=f"kq{m}")
            for idx, w in ((0, wk), (1, wq)):
                for kk in range(KT):
                    nc.tensor.matmul(
                        pair[:, idx, :],
                        w[:, kk, 128 * m : 128 * (m + 1)],
                        xT[:, kk, :],
                        start=(kk == 0),
                        stop=(kk == KT - 1),
                    )
            nc.vector.tensor_copy(kqT[:, m, :, :], pair)

        def proj_v():
            for tp in range(ST // 2):
                pair = ps_big.tile([128, 2, HGF], F32, tag="big", name=f"vp{tp}")
                for half in range(2):
                    tt = 2 * tp + half
                    for kk in range(KT):
                        nc.tensor.matmul(
                            pair[:, half, :],
                            xT[:, kk, 128 * tt : 128 * (tt + 1)],
                            wv[:, kk, :],
                            start=(kk == 0),
                            stop=(kk == KT - 1),
                        )
                nc.vector.tensor_copy(
                    v_aug[:, 2 * tp : 2 * tp + 2, :, 0:DH],
                    pair.rearrange("p a (h c) -> p a h c", c=DH),
                )

        def scores(m):
            # head pair (2m, 2m+1) as concurrent row-group matmuls; one exp
            # ACTIVATE per [128, 2, 512] pair tile
            for jt in range(ST):
                pair = ps_big.tile([128, 2, S], F32, tag="big", name=f"sc{m}{jt}")
                for sub in range(2):
                    off = 64 * sub
                    nc.tensor.matmul(
                        pair[:, sub, :],
                        kqT[off : off + 64, m, 0, 128 * jt : 128 * (jt + 1)],
                        kqT[off : off + 64, m, 1, :],
                        start=True,
                        stop=True,
                    )
                nc.scalar.activation(
                    e_sb[:, jt, 2 * m : 2 * m + 2, :], pair, AF.Exp, scale=0.125
                )

        def mm2(h):
            u = ps_o.tile([DHA, S], F32, tag="o", name=f"ups{h}")
            ups[h] = u
            for jt in range(ST):
                nc.tensor.matmul(
                    u,
                    v_aug[:, jt, h, :],
                    e_sb[:, jt, h, :],
                    start=(jt == 0),
                    stop=(jt == ST - 1),
                )
            nc.vector.tensor_copy(rrows[h], u[DH : DH + 1, :])
            nc.vector.reciprocal_approx_fast(srowFs[h], rrows[h])
            nc.vector.tensor_copy(srows[h], srowFs[h])

        def gwv(h):
            gw = ps_sel.tile([DH, 1], F32, tag="sel", name=f"gw{h}")
            for jt in range(ST):
                nc.tensor.matmul(
                    gw,
                    v_aug[:, jt, h, 0:DH],
                    wj[:, jt, h : h + 1],
                    start=(jt == 0),
                    stop=(jt == ST - 1),
                )
            off = 64 * (h % 2)
            nc.vector.tensor_copy(gwv_cols[off : off + 64, h // 2 : h // 2 + 1], gw)

        def sel_combine(h):
            sc_ps = ps_sel.tile([DH, S], F32, tag="sel", name=f"scp{h}")
            nc.tensor.matmul(
                sc_ps, ones64h, srows[h], start=True, stop=True
            )
            scSB = scpool.tile([DH, S], F16, tag="sc")
            nc.vector.tensor_copy(scSB, sc_ps)
            off = 64 * (h % 2)
            nc.vector.tensor_tensor(
                out=oT[off : off + 64, h // 2, :],
                in0=ups[h][0:DH, :],
                in1=scSB,
                op=ALU.mult,
            )

        with nc.named_scope("proj_kq0"):
            proj_kq(0)
        with nc.named_scope("scores0"):
            scores(0)

        # ---- position path (PE work slots into the exp-bound window)
        with nc.named_scope("pos_path"):
            p1ps = ps_sel.tile([4, S], F32, tag="sel")
            nc.tensor.matmul(
                p1ps, posP[:, S : S + 4], posP[:, 0:S], start=True, stop=True
            )
            p1 = sing.tile([4, S], F32R)
            nc.scalar.activation(p1, p1ps, AF.Relu, bias=sclP[:, 0:1])
            p2ps = ps_sel.tile([PD8, S], F32, tag="sel")
            nc.tensor.matmul(
                p2ps, posP[:, S + 4 :], p1[0:POS_DIM, :], start=True, stop=True
            )
            p2 = sing.tile([PD8, S], F32R)
            nc.vector.tensor_copy(p2, p2ps)  # bp2 cancels in the softmax
            phps = ps_sel.tile([HGH, S], F32, tag="sel")
            nc.tensor.matmul(phps, whP, p2, start=True, stop=True)
            wexp = sing.tile([HGH, S], F32)
            wsum = sing.tile([HGH, 1], F32)
            nc.scalar.activation(wexp, phps, AF.Exp, scale=-1.0, accum_out=wsum)
            winv = sing.tile([HGH, 1], F32)
            nc.vector.reciprocal(winv, wsum)
            gwin = sing.tile([HGH, 1], F32)
            nc.vector.tensor_mul(gwin, winv, sclP[:, 1:2])
            w_sb = sing.tile([HGH, S], F32)
            nc.vector.tensor_scalar_mul(w_sb, wexp, gwin)
            # w as columns for the w@v contraction
            wj = sing.tile([128, ST, HGH], F16)
            for jt in range(ST):
                wt = ps_sel.tile([128, HGH], F32, tag="sel", name=f"wt{jt}")
                nc.tensor.transpose(
                    wt, w_sb[:, 128 * jt : 128 * (jt + 1)], ident[0:HGH, 0:HGH]
                )
                nc.vector.tensor_copy(wj[:, jt, :], wt)

        with nc.named_scope("proj_kq1"):
            proj_kq(1)
        with nc.named_scope("proj_v"):
            proj_v()
        with nc.named_scope("scores1"):
            scores(1)
        with nc.named_scope("attn"):
            for h in range(HGH):
                gwv(h)
            mm2(0)
            mm2(1)
            sel_combine(0)
            mm2(2)
            sel_combine(1)
            mm2(3)
            sel_combine(2)
            sel_combine(3)

        # ---------------- ybias row: sum_h g*(w@v_h) @ Wo_h ----------------
        with nc.named_scope("ybias"):
            yb_ps = ps_sel.tile([1, DIM], F32, tag="sel")
            for m in range(MT):
                nc.tensor.matmul(
                    yb_ps,
                    gwv_cols[:, m : m + 1],
                    wo[:, m, :],
                    start=(m == 0),
                    stop=(m == MT - 1),
                )
            ybsb = sing.tile([1, DIM], F32)
            nc.vector.tensor_copy(ybsb, yb_ps)
            nc.sync.dma_start(out=yb_d.ap(), in_=ybsb)

        # ---------------- out-projection ----------------
        with nc.named_scope("outproj"):
            for it in range(ST):
                yps = ps_o.tile([128, DIM], F32, tag="o", name=f"yps{it}")
                for m in range(MT):
                    nc.tensor.matmul(
                        yps,
                        oT[:, m, 128 * it : 128 * (it + 1)],
                        wo[:, m, :],
                        start=(m == 0),
                        stop=(m == MT - 1),
                    )
                ysb = ypool.tile([128, DIM], F16, tag="y")
                nc.vector.tensor_copy(ysb, yps)
                nc.sync.dma_start(
                    out=y_d.ap()[128 * it : 128 * (it + 1), :], in_=ysb
                )

    nc.compile()
    return nc


def _get_program():
    if "nc" not in _CACHE:
        _CACHE["nc"] = _build_program()
    return _CACHE["nc"]


def _ktile(a, dtype=np.float16):
    # [K*128, n] -> [128, K*n] (per-partition-contiguous k-tile layout)
    k = a.shape[0] // 128
    return np.ascontiguousarray(
        a.reshape(k, 128, a.shape[1]).transpose(1, 0, 2).reshape(128, -1).astype(dtype)
    )


def _make_in_maps(inputs):
    f = lambda a: np.ascontiguousarray(np.asarray(a), dtype=np.float32)
    x = f(inputs["x"])
    pos = f(inputs["pos"])
    Wq, Wk, Wv, Wo = f(inputs["Wq"]), f(inputs["Wk"]), f(inputs["Wv"]), f(inputs["Wo"])
    Wp1, bp1 = f(inputs["Wp1"]), f(inputs["bp1"])
    Wh, gate = f(inputs["Wh"]), f(inputs["gate"])
    gfull = 1.0 / (1.0 + np.exp(-gate.astype(np.float64)))  # sigmoid on host

    wp1_pad = np.zeros((POS_DIM, 4), np.float32)
    wp1_pad[:, :POS_DIM] = Wp1
    bp1_pad = np.zeros((HGH,), np.float32)
    bp1_pad[:POS_DIM] = bp1
    Wp2 = f(inputs["Wp2"])  # [3, 64]; bp2 cancels in the softmax

    in_maps = []
    for c in range(NCORES):
        b, hg = c // 2, c % 2
        cs = slice(HGF * hg, HGF * (hg + 1))
        g = gfull[HGH * hg : HGH * (hg + 1)].astype(np.float32)
        inv1mg = (1.0 / (1.0 - g.astype(np.float64))).astype(np.float32)
        posP = np.concatenate(
            [np.ascontiguousarray(pos[b].T), wp1_pad, Wp2], axis=1
        ).astype(np.float32)
        sclP = np.zeros((HGH, 2), np.float32)
        sclP[:, 0] = bp1_pad
        sclP[:, 1] = g
        vpad = np.tile(inv1mg.astype(np.float16)[None, :], (128, ST)).reshape(128, -1)
        in_maps.append(
            {
                "xT": _ktile(x[b].T),
                "Wq": _ktile(Wq[:, cs]),
                "Wk": _ktile(Wk[:, cs]),
                "Wv": _ktile(Wv[:, cs]),
                "Wo": _ktile(Wo[cs, :]),
                "posP": posP,
                "whP": np.ascontiguousarray(Wh[:, HGH * hg : HGH * (hg + 1)]),
                "sclP": sclP,
                "vpad": np.ascontiguousarray(vpad),
            }
        )
    return in_maps


def run(inputs, trace=False):
    """Run on 8 NeuronCores; returns (out [B,S,DIM] fp32, BassKernelResults)."""
    from concourse.bass_utils import run_bass_kernel_spmd

    nc = _get_program()
    in_maps = _make_in_maps(inputs)
    res = run_bass_kernel_spmd(
        nc, in_maps, core_ids=list(range(NCORES)), trace=trace
    )
    bo = np.asarray(inputs["bo"], np.float32)
    out = np.empty((B, S, DIM), np.float32)
    for b in range(B):
        r0, r1 = res.results[2 * b], res.results[2 * b + 1]
        out[b] = (
            r0["y"].astype(np.float32)
            + r1["y"].astype(np.float32)
            + r0["yb"]
            + r1["yb"]
            + bo[None, :]
        )
    return out, res


def kernel(**inputs):
    out, _ = run(inputs, trace=False)
    return out


# revision 15
# speedup vs baseline: 1.2332x; 1.0094x over previous
"""Trainium2 Bass kernel for nn_Attention_48095043781121 (v3).

Math (reference):
    q,k,v = x@Wq, x@Wk, x@Wv          (per head h: columns [64h, 64h+64))
    A     = softmax_j(q.k^T / 8)
    p     = relu(pos@Wp1+bp1)@Wp2+bp2
    P[b,h,i,j] = softmax_j(ph_i - ph_j + bh) = softmax_j(-ph_j) = w[b,h,j]
                 (i-part, bh AND the bp2 contribution all cancel in softmax)
    attn  = ((1-g)A + gP) / rowsum               rowsum == 1 exactly
    out   = attn @ v ;  y = concat_heads(out) @ Wo + bo

Per (b,h):  y += [(1-g_h)/r] * (E @ v_h) @ Wo_h  +  [g_h * (w @ v_h)] @ Wo_h
with E = exp(S/8), r[i] = sum_j E[i,j].  The second term is a constant row
(independent of the query i) -> computed once as `yb` and added on the host
along with bo during the unshard.

Sharding: 8 cores = 4 batches x 2 head-groups (heads 0-3 / 4-7); host sums
the two partial y (+ yb rows + bo) per batch.

v3 structure (all fp16 on the PE):
  - E stored [j-part, i-free]; E@v uses v (augmented with a 1/(1-g) column)
    as stationary and E as moving, so output lands [feature-part, i-free]
    for the out-projection -- no PE transposes.
  - Scores for a head pair run as two concurrent row-group matmuls
    (K=64 at partitions 0-63 / 64-127) into the two banks of one
    [128, 2, 512] PSUM pair tile; ONE exp ACTIVATE covers the pair
    (amortizes the ~260ns ACT per-op overhead).
  - Row 64 of each head's E@v is r/(1-g); reciprocal_approx_fast + a K=1
    fp32r matmul broadcasts (1-g)/r over 64 partitions; a partition-shifted
    DVE multiply stacks the odd head into oT rows 64-127.
  - The pos-MLP runs inside the exp-bound attention window.
"""

import numpy as np
from contextlib import ExitStack

B, S, DIM, H, DH = 4, 512, 512, 8, 64
POS_DIM, PD8 = 3, 64
NCORES = 8
HGH = 4          # heads per head-group (per core)
HGF = HGH * DH   # feature columns per head-group = 256
KT = DIM // 128  # contraction tiles over model dim = 4
MT = HGF // 128  # feature tiles per head-group = 2
ST = S // 128    # token tiles = 4
DHA = DH + 1     # v columns padded: [v(64) | 1/(1-g)]
NWARM = 10       # HAM warmup matmuls bridging the input-DMA head

_CACHE = {}


def _build_program():
    import concourse.mybir as mybir
    import concourse.tile as tile
    from concourse import bacc
    from concourse.masks import make_identity

    F32 = mybir.dt.float32
    F32R = mybir.dt.float32r
    F16 = mybir.dt.float16
    AF = mybir.ActivationFunctionType
    ALU = mybir.AluOpType

    nc = bacc.Bacc(trn_type="TRN2", target_bir_lowering=False, debug=False)

    xT_d = nc.dram_tensor("xT", [128, KT * S], F16, kind="ExternalInput")
    wq_d = nc.dram_tensor("Wq", [128, KT * HGF], F16, kind="ExternalInput")
    wk_d = nc.dram_tensor("Wk", [128, KT * HGF], F16, kind="ExternalInput")
    wv_d = nc.dram_tensor("Wv", [128, KT * HGF], F16, kind="ExternalInput")
    wo_d = nc.dram_tensor("Wo", [128, MT * DIM], F16, kind="ExternalInput")
    # posP: [posT(512) | Wp1 padded to 4 | Wp2(64)]
    posP_d = nc.dram_tensor("posP", [POS_DIM, S + 4 + PD8], F32R, kind="ExternalInput")
    whP_d = nc.dram_tensor("whP", [PD8, HGH], F32R, kind="ExternalInput")
    # sclP: [bp1 | g]
    sclP_d = nc.dram_tensor("sclP", [HGH, 2], F32, kind="ExternalInput")
    vpad_d = nc.dram_tensor("vpad", [128, ST * HGH], F16, kind="ExternalInput")
    y_d = nc.dram_tensor("y", [S, DIM], F16, kind="ExternalOutput")
    yb_d = nc.dram_tensor("yb", [1, DIM], F32, kind="ExternalOutput")

    with tile.TileContext(nc) as tc, ExitStack() as ctx:
        sing = ctx.enter_context(tc.tile_pool(name="sing", bufs=1))
        scpool = ctx.enter_context(tc.tile_pool(name="scpool", bufs=2))
        ypool = ctx.enter_context(tc.tile_pool(name="ypool", bufs=2))
        # PSUM: 8 banks = ps_big 2x2 + ps_o 3x1 + ps_sel 1x1
        ps_big = ctx.enter_context(tc.tile_pool(name="ps_big", bufs=2, space="PSUM"))
        ps_o = ctx.enter_context(tc.tile_pool(name="ps_o", bufs=3, space="PSUM"))
        ps_sel = ctx.enter_context(tc.tile_pool(name="ps_sel", bufs=1, space="PSUM"))

        # ---------------- input DMAs (3 queues) ----------------
        xT = sing.tile([128, KT, S], F16)
        wq = sing.tile([128, KT, HGF], F16)
        wk = sing.tile([128, KT, HGF], F16)
        wv = sing.tile([128, KT, HGF], F16)
        wo = sing.tile([128, MT, DIM], F16)
        v_aug = sing.tile([128, ST, HGH, DHA], F16)
        posP = sing.tile([POS_DIM, S + 4 + PD8], F32R)
        whP = sing.tile([PD8, HGH], F32R)
        sclP = sing.tile([HGH, 2], F32)

        xT_r = xT_d.ap()
        nc.sync.dma_start(out=xT[:, 0:2, :], in_=xT_r[:, 0 : 2 * S])
        nc.scalar.dma_start(out=wq, in_=wq_d.ap())
        nc.scalar.dma_start(out=xT[:, 2:KT, :], in_=xT_r[:, 2 * S : KT * S])
        nc.gpsimd.dma_start(out=wk, in_=wk_d.ap())
        nc.gpsimd.dma_start(out=wv, in_=wv_d.ap())
        nc.gpsimd.dma_start(out=posP, in_=posP_d.ap())
        nc.gpsimd.dma_start(out=whP, in_=whP_d.ap())
        nc.gpsimd.dma_start(out=sclP, in_=sclP_d.ap())
        # aug column of v (1/(1-g) per head) straight from the host
        nc.gpsimd.dma_start(
            out=v_aug[:, :, :, DH : DH + 1],
            in_=vpad_d.ap().rearrange("p (t h) -> p t h", h=HGH)[:, :, :, None],
        )
        nc.gpsimd.dma_start(out=wo, in_=wo_d.ap())

        # ---------------- constants + HAM warmup ----------------
        warm = sing.tile([128, 512], F16)
        nc.vector.memset(warm, 0.25)
        ones1_f = sing.tile([1, DH], F32)
        nc.vector.memset(ones1_f, 1.0)
        ones64h = sing.tile([1, DH], F16)
        nc.vector.tensor_copy(ones64h, ones1_f)
        ident = sing.tile([128, 128], F32)
        make_identity(nc, ident)
        with nc.named_scope("warmup"):
            for _ in range(NWARM):
                wps = ps_big.tile([128, 512], F32, tag="big")
                nc.tensor.matmul(wps, warm[:, 0:128], warm, start=True, stop=True)

        # ---------------- projections / scores / attention ----------------
        # kqT[:, m, 0, :] = k features (m-block), kqT[:, m, 1, :] = q
        kqT = sing.tile([128, MT, 2, S], F16)
        e_sb = sing.tile([128, ST, HGH, S], F16)
        oT = sing.tile([128, MT, S], F16)
        gwv_cols = sing.tile([128, MT], F16)
        srows = [sing.tile([1, S], F16, name=f"srow{h}") for h in range(HGH)]
        srowFs = [sing.tile([1, S], F32, name=f"srowF{h}") for h in range(HGH)]
        rrows = [sing.tile([1, S], F32, name=f"rrow{h}") for h in range(HGH)]
        ups = [None] * HGH

        def proj_kq(m):
            pair = ps_big.tile([128, 2, S], F32, tag="big", name=f"kq{m}")
            for kks in (range(0, 2), range(2, KT)):
                for idx, w in ((0, wk), (1, wq)):
                    for kk in kks:
                        nc.tensor.matmul(
                            pair[:, idx, :],
                            w[:, kk, 128 * m : 128 * (m + 1)],
                            xT[:, kk, :],
                            start=(kk == 0),
                            stop=(kk == KT - 1),
                        )
            nc.vector.tensor_copy(kqT[:, m, :, :], pair)

        def proj_v():
            for tp in range(ST // 2):
                pair = ps_big.tile([128, 2, HGF], F32, tag="big", name=f"vp{tp}")
                for half in range(2):
                    tt = 2 * tp + half
                    for kk in range(KT):
                        nc.tensor.matmul(
                            pair[:, half, :],
                            xT[:, kk, 128 * tt : 128 * (tt + 1)],
                            wv[:, kk, :],
                            start=(kk == 0),
                            stop=(kk == KT - 1),
                        )
                nc.vector.tensor_copy(
                    v_aug[:, 2 * tp : 2 * tp + 2, :, 0:DH],
                    pair.rearrange("p a (h c) -> p a h c", c=DH),
                )

        def scores(m):
            # head pair (2m, 2m+1) as concurrent row-group matmuls; one exp
            # ACTIVATE per [128, 2, 512] pair tile
            for jt in range(ST):
                pair = ps_big.tile([128, 2, S], F32, tag="big", name=f"sc{m}{jt}")
                for sub in range(2):
                    off = 64 * sub
                    nc.tensor.matmul(
                        pair[:, sub, :],
                        kqT[off : off + 64, m, 0, 128 * jt : 128 * (jt + 1)],
                        kqT[off : off + 64, m, 1, :],
                        start=True,
                        stop=True,
                    )
                nc.scalar.activation(
                    e_sb[:, jt, 2 * m : 2 * m + 2, :], pair, AF.Exp, scale=0.125
                )

        def mm2(h):
            u = ps_o.tile([DHA, S], F32, tag="o", name=f"ups{h}")
            ups[h] = u
            for jt in range(ST):
                nc.tensor.matmul(
                    u,
                    v_aug[:, jt, h, :],
                    e_sb[:, jt, h, :],
                    start=(jt == 0),
                    stop=(jt == ST - 1),
                )
            nc.vector.tensor_copy(rrows[h], u[DH : DH + 1, :])
            nc.vector.reciprocal_approx_fast(srowFs[h], rrows[h])
            nc.vector.tensor_copy(srows[h], srowFs[h])

        def gwv(h):
            gw = ps_sel.tile([DH, 1], F32, tag="sel", name=f"gw{h}")
            for jt in range(ST):
                nc.tensor.matmul(
                    gw,
                    v_aug[:, jt, h, 0:DH],
                    wj[:, jt, h : h + 1],
                    start=(jt == 0),
                    stop=(jt == ST - 1),
                )
            off = 64 * (h % 2)
            nc.vector.tensor_copy(gwv_cols[off : off + 64, h // 2 : h // 2 + 1], gw)

        def sel_combine(h):
            sc_ps = ps_sel.tile([DH, S], F32, tag="sel", name=f"scp{h}")
            nc.tensor.matmul(
                sc_ps, ones64h, srows[h], start=True, stop=True
            )
            scSB = scpool.tile([DH, S], F16, tag="sc")
            nc.vector.tensor_copy(scSB, sc_ps)
            off = 64 * (h % 2)
            nc.vector.tensor_tensor(
                out=oT[off : off + 64, h // 2, :],
                in0=ups[h][0:DH, :],
                in1=scSB,
                op=ALU.mult,
            )

        with nc.named_scope("proj_kq0"):
            proj_kq(0)
        with nc.named_scope("scores0"):
            scores(0)

        # ---- position path (PE work slots into the exp-bound window)
        with nc.named_scope("pos_path"):
            p1ps = ps_sel.tile([4, S], F32, tag="sel")
            nc.tensor.matmul(
                p1ps, posP[:, S : S + 4], posP[:, 0:S], start=True, stop=True
            )
            p1 = sing.tile([4, S], F32R)
            nc.scalar.activation(p1, p1ps, AF.Relu, bias=sclP[:, 0:1])
            p2ps = ps_sel.tile([PD8, S], F32, tag="sel")
            nc.tensor.matmul(
                p2ps, posP[:, S + 4 :], p1[0:POS_DIM, :], start=True, stop=True
            )
            p2 = sing.tile([PD8, S], F32R)
            nc.vector.tensor_copy(p2, p2ps)  # bp2 cancels in the softmax
            phps = ps_sel.tile([HGH, S], F32, tag="sel")
            nc.tensor.matmul(phps, whP, p2, start=True, stop=True)
            wexp = sing.tile([HGH, S], F32)
            wsum = sing.tile([HGH, 1], F32)
            nc.scalar.activation(wexp, phps, AF.Exp, scale=-1.0, accum_out=wsum)
            winv = sing.tile([HGH, 1], F32)
            nc.vector.reciprocal(winv, wsum)
            gwin = sing.tile([HGH, 1], F32)
            nc.vector.tensor_mul(gwin, winv, sclP[:, 1:2])
            w_sb = sing.tile([HGH, S], F32)
            nc.vector.tensor_scalar_mul(w_sb, wexp, gwin)
            # w as columns for the w@v contraction
            wj = sing.tile([128, ST, HGH], F16)
            for jt in range(ST):
                wt = ps_sel.tile([128, HGH], F32, tag="sel", name=f"wt{jt}")
                nc.tensor.transpose(
                    wt, w_sb[:, 128 * jt : 128 * (jt + 1)], ident[0:HGH, 0:HGH]
                )
                nc.vector.tensor_copy(wj[:, jt, :], wt)

        with nc.named_scope("proj_kq1"):
            proj_kq(1)
        with nc.named_scope("proj_v"):
            proj_v()
        with nc.named_scope("scores1"):
            scores(1)
        with nc.named_scope("gwv_ybias"):
            for h in range(HGH):
                gwv(h)
            yb_ps = ps_sel.tile([1, DIM], F32, tag="sel")
            for m in range(MT):
                nc.tensor.matmul(
                    yb_ps,
                    gwv_cols[:, m : m + 1],
                    wo[:, m, :],
                    start=(m == 0),
                    stop=(m == MT - 1),
                )
            ybsb = sing.tile([1, DIM], F32)
            nc.vector.tensor_copy(ybsb, yb_ps)
            nc.sync.dma_start(out=yb_d.ap(), in_=ybsb)

        with nc.named_scope("attn"):
            mm2(0)
            mm2(1)
            sel_combine(0)
            mm2(2)
            sel_combine(1)
            mm2(3)
            sel_combine(2)
            sel_combine(3)

        # ---------------- out-projection ----------------
        with nc.named_scope("outproj"):
            for it in range(ST):
                yps = ps_o.tile([128, DIM], F32, tag="o", name=f"yps{it}")
                for m in range(MT):
                    nc.tensor.matmul(
                        yps,
                        oT[:, m, 128 * it : 128 * (it + 1)],
                        wo[:, m, :],
                        start=(m == 0),
                        stop=(m == MT - 1),
                    )
                ysb = ypool.tile([128, DIM], F16, tag="y")
                nc.vector.tensor_copy(ysb, yps)
                nc.sync.dma_start(
                    out=y_d.ap()[128 * it : 128 * (it + 1), :], in_=ysb
                )

    nc.compile()
    return nc


def _get_program():
    if "nc" not in _CACHE:
        _CACHE["nc"] = _build_program()
    return _CACHE["nc"]


def _ktile(a, dtype=np.float16):
    # [K*128, n] -> [128, K*n] (per-partition-contiguous k-tile layout)
    k = a.shape[0] // 128
    return np.ascontiguousarray(
        a.reshape(k, 128, a.shape[1]).transpose(1, 0, 2).reshape(128, -1).astype(dtype)
    )


def _make_in_maps(inputs):
    f = lambda a: np.ascontiguousarray(np.asarray(a), dtype=np.float32)
    x = f(inputs["x"])
    pos = f(inputs["pos"])
    Wq, Wk, Wv, Wo = f(inputs["Wq"]), f(inputs["Wk"]), f(inputs["Wv"]), f(inputs["Wo"])
    Wp1, bp1 = f(inputs["Wp1"]), f(inputs["bp1"])
    Wh, gate = f(inputs["Wh"]), f(inputs["gate"])
    gfull = 1.0 / (1.0 + np.exp(-gate.astype(np.float64)))  # sigmoid on host

    wp1_pad = np.zeros((POS_DIM, 4), np.float32)
    wp1_pad[:, :POS_DIM] = Wp1
    bp1_pad = np.zeros((HGH,), np.float32)
    bp1_pad[:POS_DIM] = bp1
    Wp2 = f(inputs["Wp2"])  # [3, 64]; bp2 cancels in the softmax

    in_maps = []
    for c in range(NCORES):
        b, hg = c // 2, c % 2
        cs = slice(HGF * hg, HGF * (hg + 1))
        g = gfull[HGH * hg : HGH * (hg + 1)].astype(np.float32)
        inv1mg = (1.0 / (1.0 - g.astype(np.float64))).astype(np.float32)
        posP = np.concatenate(
            [np.ascontiguousarray(pos[b].T), wp1_pad, Wp2], axis=1
        ).astype(np.float32)
        sclP = np.zeros((HGH, 2), np.float32)
        sclP[:, 0] = bp1_pad
        sclP[:, 1] = g
        vpad = np.tile(inv1mg.astype(np.float16)[None, :], (128, ST)).reshape(128, -1)
        in_maps.append(
            {
                "xT": _ktile(x[b].T),
                "Wq": _ktile(Wq[:, cs]),
                "Wk": _ktile(Wk[:, cs]),
                "Wv": _ktile(Wv[:, cs]),
                "Wo": _ktile(Wo[cs, :]),
                "posP": posP,
                "whP": np.ascontiguousarray(Wh[:, HGH * hg : HGH * (hg + 1)]),
                "sclP": sclP,
                "vpad": np.ascontiguousarray(vpad),
            }
        )
    return in_maps


def run(inputs, trace=False):
    """Run on 8 NeuronCores; returns (out [B,S,DIM] fp32, BassKernelResults)."""
    from concourse.bass_utils import run_bass_kernel_spmd

    nc = _get_program()
    in_maps = _make_in_maps(inputs)
    res = run_bass_kernel_spmd(
        nc, in_maps, core_ids=list(range(NCORES)), trace=trace
    )
    bo = np.asarray(inputs["bo"], np.float32)
    out = np.empty((B, S, DIM), np.float32)
    for b in range(B):
        r0, r1 = res.results[2 * b], res.results[2 * b + 1]
        out[b] = (
            r0["y"].astype(np.float32)
            + r1["y"].astype(np.float32)
            + r0["yb"]
            + r1["yb"]
            + bo[None, :]
        )
    return out, res


def kernel(**inputs):
    out, _ = run(inputs, trace=False)
    return out


# revision 16
# speedup vs baseline: 1.3218x; 1.0718x over previous
"""Trainium2 Bass kernel for nn_Attention_48095043781121 (v5).

Math (reference):
    q,k,v = x@Wq, x@Wk, x@Wv          (per head h: columns [64h, 64h+64))
    A     = softmax_j(q.k^T / 8)
    p     = relu(pos@Wp1+bp1)@Wp2+bp2
    P[b,h,i,j] = softmax_j(ph_i - ph_j + bh) = softmax_j(-ph_j) = w[b,h,j]
                 (i-part, bh AND the bp2 contribution all cancel in softmax)
    attn  = ((1-g)A + gP) / rowsum               rowsum == 1 exactly
    out   = attn @ v ;  y = concat_heads(out) @ Wo + bo

Per (b,h):  y += [(1-g_h)/r] * (E @ v_h) @ Wo_h  +  [g_h * (w @ v_h)] @ Wo_h
with E = exp(S/8), r[i] = sum_j E[i,j].  The second term is a constant row
(independent of the query i) -> computed once as `yb` and added on the host
along with bo during the unshard.

Sharding: 8 cores = 4 batches x 2 head-groups (heads 0-3 / 4-7); host sums
the two partial y (+ yb rows + bo) per batch.

v5 structure (all fp16 on the PE):
  - E stored [j-part, i-free]; E@v uses v (augmented with a 1/(1-g) column)
    as stationary and E as moving, so output lands [feature-part, i-free]
    for the out-projection -- no PE transposes.
  - Scores for a head pair run as two concurrent row-group matmuls
    (K=64 at partitions 0-63 / 64-127) into the two banks of one
    [128, 2, 512] PSUM pair tile; ONE exp ACTIVATE covers the pair.
    Scores own ps_big so the exp-ring never blocks projection matmuls.
  - Row 64 of each head's E@v is r/(1-g); a partition-shift copy +
    reciprocal_approx_fast + K=1 fp16 matmul broadcasts (1-g)/r over 64
    partitions; a partition-shifted DVE multiply stacks the odd head into
    oT rows 64-127.
  - PSUM->SBUF evacuations are split between DVE and ACT (ACT is idle
    outside the exp window); the pos-MLP runs inside the exp-bound window.
"""

import numpy as np
from contextlib import ExitStack

B, S, DIM, H, DH = 4, 512, 512, 8, 64
POS_DIM, PD8 = 3, 64
NCORES = 8
HGH = 4          # heads per head-group (per core)
HGF = HGH * DH   # feature columns per head-group = 256
KT = DIM // 128  # contraction tiles over model dim = 4
MT = HGF // 128  # feature tiles per head-group = 2
ST = S // 128    # token tiles = 4
DHA = DH + 1     # v columns padded: [v(64) | 1/(1-g)]
NWARM = 8        # HAM warmup matmuls bridging the input-DMA head

_CACHE = {}


def _build_program():
    import concourse.mybir as mybir
    import concourse.tile as tile
    from concourse import bacc
    from concourse.masks import make_identity

    F32 = mybir.dt.float32
    F32R = mybir.dt.float32r
    F16 = mybir.dt.float16
    AF = mybir.ActivationFunctionType
    ALU = mybir.AluOpType

    nc = bacc.Bacc(trn_type="TRN2", target_bir_lowering=False, debug=False)

    xT_d = nc.dram_tensor("xT", [128, KT * S], F16, kind="ExternalInput")
    wq_d = nc.dram_tensor("Wq", [128, KT * HGF], F16, kind="ExternalInput")
    wk_d = nc.dram_tensor("Wk", [128, KT * HGF], F16, kind="ExternalInput")
    wv_d = nc.dram_tensor("Wv", [128, KT * HGF], F16, kind="ExternalInput")
    wo_d = nc.dram_tensor("Wo", [128, MT * DIM], F16, kind="ExternalInput")
    # posP: [posT(512) | Wp1 padded to 4 | Wp2(64)]
    posP_d = nc.dram_tensor("posP", [POS_DIM, S + 4 + PD8], F32R, kind="ExternalInput")
    whP_d = nc.dram_tensor("whP", [PD8, HGH], F32R, kind="ExternalInput")
    # sclP: [bp1 | g]
    sclP_d = nc.dram_tensor("sclP", [HGH, 2], F32, kind="ExternalInput")
    vpad_d = nc.dram_tensor("vpad", [128, ST * HGH], F16, kind="ExternalInput")
    y_d = nc.dram_tensor("y", [S, DIM], F16, kind="ExternalOutput")
    yb_d = nc.dram_tensor("yb", [1, DIM], F32, kind="ExternalOutput")

    with tile.TileContext(nc) as tc, ExitStack() as ctx:
        sing = ctx.enter_context(tc.tile_pool(name="sing", bufs=1))
        scpool = ctx.enter_context(tc.tile_pool(name="scpool", bufs=2))
        ypool = ctx.enter_context(tc.tile_pool(name="ypool", bufs=2))
        # PSUM: 8 banks = ps_big 2x2 (score pairs only) + ps_o 3x1 + ps_sel 1x1
        ps_big = ctx.enter_context(tc.tile_pool(name="ps_big", bufs=2, space="PSUM"))
        ps_o = ctx.enter_context(tc.tile_pool(name="ps_o", bufs=3, space="PSUM"))
        ps_sel = ctx.enter_context(tc.tile_pool(name="ps_sel", bufs=1, space="PSUM"))

        # ---------------- input DMAs (3 queues) ----------------
        xT = sing.tile([128, KT, S], F16)
        wq = sing.tile([128, KT, HGF], F16)
        wk = sing.tile([128, KT, HGF], F16)
        wv = sing.tile([128, KT, HGF], F16)
        wo = sing.tile([128, MT, DIM], F16)
        v_aug = sing.tile([128, ST, HGH, DHA], F16)
        posP = sing.tile([POS_DIM, S + 4 + PD8], F32R)
        whP = sing.tile([PD8, HGH], F32R)
        sclP = sing.tile([HGH, 2], F32)

        xT_r = xT_d.ap()
        nc.sync.dma_start(out=xT[:, 0:2, :], in_=xT_r[:, 0 : 2 * S])
        nc.scalar.dma_start(out=wq, in_=wq_d.ap())
        nc.scalar.dma_start(out=xT[:, 2:KT, :], in_=xT_r[:, 2 * S : KT * S])
        nc.gpsimd.dma_start(out=wk, in_=wk_d.ap())
        nc.gpsimd.dma_start(out=wv, in_=wv_d.ap())
        nc.gpsimd.dma_start(out=posP, in_=posP_d.ap())
        nc.gpsimd.dma_start(out=whP, in_=whP_d.ap())
        nc.gpsimd.dma_start(out=sclP, in_=sclP_d.ap())
        # aug column of v (1/(1-g) per head) straight from the host
        nc.gpsimd.dma_start(
            out=v_aug[:, :, :, DH : DH + 1],
            in_=vpad_d.ap().rearrange("p (t h) -> p t h", h=HGH)[:, :, :, None],
        )
        nc.gpsimd.dma_start(out=wo, in_=wo_d.ap())

        # ---------------- constants + HAM warmup ----------------
        warm = sing.tile([128, 512], F16)
        nc.vector.memset(warm, 0.25)
        ones1_f = sing.tile([1, DH], F32)
        nc.vector.memset(ones1_f, 1.0)
        ones64h = sing.tile([1, DH], F16)
        nc.vector.tensor_copy(ones64h, ones1_f)
        ident = sing.tile([128, 128], F32)
        make_identity(nc, ident)
        with nc.named_scope("warmup"):
            for _ in range(NWARM):
                wps = ps_o.tile([128, 512], F32, tag="o")
                nc.tensor.matmul(wps, warm[:, 0:128], warm, start=True, stop=True)

        # ---------------- tiles ----------------
        # kqT[:, m, 0, :] = k features (m-block), kqT[:, m, 1, :] = q
        kqT = sing.tile([128, MT, 2, S], F16)
        e_sb = sing.tile([128, ST, HGH, S], F16)
        oT = sing.tile([128, MT, S], F16)
        gwv_cols = sing.tile([128, MT], F16)
        srows = [sing.tile([1, S], F16, name=f"srow{h}") for h in range(HGH)]
        srowFs = [sing.tile([1, S], F32, name=f"srowF{h}") for h in range(HGH)]
        rrows = [sing.tile([1, S], F32, name=f"rrow{h}") for h in range(HGH)]
        ups = [None] * HGH

        def proj_kq(m, evac_engines):
            # k and q as single-bank tiles; kk 0-1 first so m=0 can start
            # on the first xT half
            kps = ps_o.tile([128, S], F32, tag="o", name=f"kp{m}")
            qps = ps_o.tile([128, S], F32, tag="o", name=f"qp{m}")
            for kks in (range(0, 2), range(2, KT)):
                for idx, w, ps in ((0, wk, kps), (1, wq, qps)):
                    for kk in kks:
                        nc.tensor.matmul(
                            ps,
                            w[:, kk, 128 * m : 128 * (m + 1)],
                            xT[:, kk, :],
                            start=(kk == 0),
                            stop=(kk == KT - 1),
                        )
            for idx, ps in ((0, kps), (1, qps)):
                if evac_engines[idx] == "act":
                    nc.scalar.activation(kqT[:, m, idx, :], ps, AF.Copy)
                else:
                    nc.vector.tensor_copy(kqT[:, m, idx, :], ps)

        def proj_v():
            for tp in range(ST // 2):
                pair = ps_o.tile([128, 2, HGF], F32, tag="o", name=f"vp{tp}")
                for half in range(2):
                    tt = 2 * tp + half
                    for kk in range(KT):
                        nc.tensor.matmul(
                            pair[:, half, :],
                            xT[:, kk, 128 * tt : 128 * (tt + 1)],
                            wv[:, kk, :],
                            start=(kk == 0),
                            stop=(kk == KT - 1),
                        )
                nc.vector.tensor_copy(
                    v_aug[:, 2 * tp : 2 * tp + 2, :, 0:DH],
                    pair.rearrange("p a (h c) -> p a h c", c=DH),
                )

        def scores(m):
            # head pair (2m, 2m+1) as concurrent row-group matmuls; one exp
            # ACTIVATE per [128, 2, 512] pair tile
            for jt in range(ST):
                pair = ps_big.tile([128, 2, S], F32, tag="big", name=f"sc{m}{jt}")
                for sub in range(2):
                    off = 64 * sub
                    nc.tensor.matmul(
                        pair[:, sub, :],
                        kqT[off : off + 64, m, 0, 128 * jt : 128 * (jt + 1)],
                        kqT[off : off + 64, m, 1, :],
                        start=True,
                        stop=True,
                    )
                nc.scalar.activation(
                    e_sb[:, jt, 2 * m : 2 * m + 2, :], pair, AF.Exp, scale=0.125
                )

        def mm2(h, row_engine):
            u = ps_o.tile([DHA, S], F32, tag="o", name=f"ups{h}")
            ups[h] = u
            for jt in range(ST):
                nc.tensor.matmul(
                    u,
                    v_aug[:, jt, h, :],
                    e_sb[:, jt, h, :],
                    start=(jt == 0),
                    stop=(jt == ST - 1),
                )
            # r/(1-g) row -> partition 0 -> 1/x -> f16 for the broadcast MM
            if row_engine == "act":
                nc.scalar.activation(rrows[h], u[DH : DH + 1, :], AF.Copy)
            else:
                nc.vector.tensor_copy(rrows[h], u[DH : DH + 1, :])
            nc.vector.reciprocal_approx_fast(srowFs[h], rrows[h])
            nc.vector.tensor_copy(srows[h], srowFs[h])

        def gwv(h):
            gw = ps_sel.tile([DH, 1], F32, tag="sel", name=f"gw{h}")
            for jt in range(ST):
                nc.tensor.matmul(
                    gw,
                    v_aug[:, jt, h, 0:DH],
                    wj[:, jt, h : h + 1],
                    start=(jt == 0),
                    stop=(jt == ST - 1),
                )
            off = 64 * (h % 2)
            nc.vector.tensor_copy(gwv_cols[off : off + 64, h // 2 : h // 2 + 1], gw)

        def sel_combine(h, evac_engine):
            sc_ps = ps_sel.tile([DH, S], F32, tag="sel", name=f"scp{h}")
            nc.tensor.matmul(sc_ps, ones64h, srows[h], start=True, stop=True)
            scSB = scpool.tile([DH, S], F16, tag="sc")
            if evac_engine == "act":
                nc.scalar.activation(scSB, sc_ps, AF.Copy)
            else:
                nc.vector.tensor_copy(scSB, sc_ps)
            off = 64 * (h % 2)
            nc.vector.tensor_tensor(
                out=oT[off : off + 64, h // 2, :],
                in0=ups[h][0:DH, :],
                in1=scSB,
                op=ALU.mult,
            )

        with nc.named_scope("proj_kq0"):
            proj_kq(0, ("act", "vec"))
        with nc.named_scope("scores0"):
            scores(0)

        # ---- position path (PE work slots into the exp-bound window)
        with nc.named_scope("pos_path"):
            p1ps = ps_sel.tile([4, S], F32, tag="sel")
            nc.tensor.matmul(
                p1ps, posP[:, S : S + 4], posP[:, 0:S], start=True, stop=True
            )
            p1 = sing.tile([4, S], F32R)
            nc.scalar.activation(p1, p1ps, AF.Relu, bias=sclP[:, 0:1])
            p2ps = ps_sel.tile([PD8, S], F32, tag="sel")
            nc.tensor.matmul(
                p2ps, posP[:, S + 4 :], p1[0:POS_DIM, :], start=True, stop=True
            )
            p2 = sing.tile([PD8, S], F32R)
            nc.vector.tensor_copy(p2, p2ps)  # bp2 cancels in the softmax
            phps = ps_sel.tile([HGH, S], F32, tag="sel")
            nc.tensor.matmul(phps, whP, p2, start=True, stop=True)
            wexp = sing.tile([HGH, S], F32)
            wsum = sing.tile([HGH, 1], F32)
            nc.scalar.activation(wexp, phps, AF.Exp, scale=-1.0, accum_out=wsum)
            winv = sing.tile([HGH, 1], F32)
            nc.vector.reciprocal(winv, wsum)
            gwin = sing.tile([HGH, 1], F32)
            nc.vector.tensor_mul(gwin, winv, sclP[:, 1:2])
            w_sb = sing.tile([HGH, S], F32)
            nc.vector.tensor_scalar_mul(w_sb, wexp, gwin)
            # w as columns for the w@v contraction
            wj = sing.tile([128, ST, HGH], F16)
            for jt in range(ST):
                wt = ps_sel.tile([128, HGH], F32, tag="sel", name=f"wt{jt}")
                nc.tensor.transpose(
                    wt, w_sb[:, 128 * jt : 128 * (jt + 1)], ident[0:HGH, 0:HGH]
                )
                nc.vector.tensor_copy(wj[:, jt, :], wt)

        with nc.named_scope("proj_kq1"):
            proj_kq(1, ("vec", "vec"))
        with nc.named_scope("proj_v"):
            proj_v()
        with nc.named_scope("scores1"):
            scores(1)
        with nc.named_scope("gwv_ybias"):
            for h in range(HGH):
                gwv(h)
            yb_ps = ps_sel.tile([1, DIM], F32, tag="sel")
            for m in range(MT):
                nc.tensor.matmul(
                    yb_ps,
                    gwv_cols[:, m : m + 1],
                    wo[:, m, :],
                    start=(m == 0),
                    stop=(m == MT - 1),
                )
            ybsb = sing.tile([1, DIM], F32)
            nc.vector.tensor_copy(ybsb, yb_ps)
            nc.sync.dma_start(out=yb_d.ap(), in_=ybsb)

        with nc.named_scope("attn"):
            mm2(0, "vec")
            mm2(1, "vec")
            sel_combine(0, "vec")
            mm2(2, "act")
            sel_combine(1, "vec")
            mm2(3, "act")
            sel_combine(2, "act")
            sel_combine(3, "act")

        # ---------------- out-projection ----------------
        with nc.named_scope("outproj"):
            for it in range(ST):
                yps = ps_o.tile([128, DIM], F32, tag="o", name=f"yps{it}")
                for m in range(MT):
                    nc.tensor.matmul(
                        yps,
                        oT[:, m, 128 * it : 128 * (it + 1)],
                        wo[:, m, :],
                        start=(m == 0),
                        stop=(m == MT - 1),
                    )
                ysb = ypool.tile([128, DIM], F16, tag="y")
                nc.scalar.activation(ysb, yps, AF.Copy)
                nc.sync.dma_start(
                    out=y_d.ap()[128 * it : 128 * (it + 1), :], in_=ysb
                )

    nc.compile()
    return nc


def _get_program():
    if "nc" not in _CACHE:
        _CACHE["nc"] = _build_program()
    return _CACHE["nc"]


def _ktile(a, dtype=np.float16):
    # [K*128, n] -> [128, K*n] (per-partition-contiguous k-tile layout)
    k = a.shape[0] // 128
    return np.ascontiguousarray(
        a.reshape(k, 128, a.shape[1]).transpose(1, 0, 2).reshape(128, -1).astype(dtype)
    )


def _make_in_maps(inputs):
    f = lambda a: np.ascontiguousarray(np.asarray(a), dtype=np.float32)
    x = f(inputs["x"])
    pos = f(inputs["pos"])
    Wq, Wk, Wv, Wo = f(inputs["Wq"]), f(inputs["Wk"]), f(inputs["Wv"]), f(inputs["Wo"])
    Wp1, bp1 = f(inputs["Wp1"]), f(inputs["bp1"])
    Wh, gate = f(inputs["Wh"]), f(inputs["gate"])
    gfull = 1.0 / (1.0 + np.exp(-gate.astype(np.float64)))  # sigmoid on host

    wp1_pad = np.zeros((POS_DIM, 4), np.float32)
    wp1_pad[:, :POS_DIM] = Wp1
    bp1_pad = np.zeros((HGH,), np.float32)
    bp1_pad[:POS_DIM] = bp1
    Wp2 = f(inputs["Wp2"])  # [3, 64]; bp2 cancels in the softmax

    in_maps = []
    for c in range(NCORES):
        b, hg = c // 2, c % 2
        cs = slice(HGF * hg, HGF * (hg + 1))
        g = gfull[HGH * hg : HGH * (hg + 1)].astype(np.float32)
        inv1mg = (1.0 / (1.0 - g.astype(np.float64))).astype(np.float32)
        posP = np.concatenate(
            [np.ascontiguousarray(pos[b].T), wp1_pad, Wp2], axis=1
        ).astype(np.float32)
        sclP = np.zeros((HGH, 2), np.float32)
        sclP[:, 0] = bp1_pad
        sclP[:, 1] = g
        vpad = np.tile(inv1mg.astype(np.float16)[None, :], (128, ST)).reshape(128, -1)
        in_maps.append(
            {
                "xT": _ktile(x[b].T),
                "Wq": _ktile(Wq[:, cs]),
                "Wk": _ktile(Wk[:, cs]),
                "Wv": _ktile(Wv[:, cs]),
                "Wo": _ktile(Wo[cs, :]),
                "posP": posP,
                "whP": np.ascontiguousarray(Wh[:, HGH * hg : HGH * (hg + 1)]),
                "sclP": sclP,
                "vpad": np.ascontiguousarray(vpad),
            }
        )
    return in_maps


def run(inputs, trace=False):
    """Run on 8 NeuronCores; returns (out [B,S,DIM] fp32, BassKernelResults)."""
    from concourse.bass_utils import run_bass_kernel_spmd

    nc = _get_program()
    in_maps = _make_in_maps(inputs)
    res = run_bass_kernel_spmd(
        nc, in_maps, core_ids=list(range(NCORES)), trace=trace
    )
    bo = np.asarray(inputs["bo"], np.float32)
    out = np.empty((B, S, DIM), np.float32)
    for b in range(B):
        r0, r1 = res.results[2 * b], res.results[2 * b + 1]
        out[b] = (
            r0["y"].astype(np.float32)
            + r1["y"].astype(np.float32)
            + r0["yb"]
            + r1["yb"]
            + bo[None, :]
        )
    return out, res


def kernel(**inputs):
    out, _ = run(inputs, trace=False)
    return out
